# revision 14
# baseline (speedup 1.0000x reference)
"""Trainium2 Bass kernel for nn_EpiSIGNetV3_NoSIG.

Sharding: data-parallel over batch B=16 across 8 NeuronCores (2 samples
per core). On-chip, activations are feature-major ([features->partitions,
B*N rows->free]) so every GEMM contraction sits on PE partitions with
weights stationary. Matmuls run in float32r (1 cyc/row); the temporal
encoder runs in bf16 (its output feeds a LayerNorm). Param-only folds
(BN affine, softmax(scale_w), sigmoid(geo_w)/sigmoid(hw_ratio), decay
exp, bias merges) are done on the host; x-dependent work is on device.
"""

import contextlib

import numpy as np

import concourse.bacc as bacc
import concourse.bass as bass
import concourse.mybir as mybir
import concourse.tile as tile
from concourse.bass_utils import run_bass_kernel_spmd

F32 = mybir.dt.float32
F32R = mybir.dt.float32r
BF16 = mybir.dt.bfloat16
AF = mybir.ActivationFunctionType
ALU = mybir.AluOpType

B, T, N = 16, 64, 512
HIDDEN, HEADS, FC, KW = 256, 4, 16, 3
HORIZON, HWIN = 24, 4
H2, H4 = HIDDEN // 2, HIDDEN // 4
HD = HIDDEN // HEADS
NCORES = 8
S = B // NCORES          # samples per core
R = S * N                # rows per core (1024)
FH = R // 2              # free-dim half (512)
NF = FC * T              # encoder features (1024)
NCH = NF // 128          # encoder feature chunks (8)
EPS = 1e-5


def _np(a):
    return np.ascontiguousarray(np.asarray(a, dtype=np.float32))


def _sigmoid(x):
    return 1.0 / (1.0 + np.exp(-x))


def _scalar(x):
    return float(np.asarray(x, dtype=np.float32).reshape(-1)[0])


def _fidx(t, c):                   # t-major encoder feature index
    return t * FC + c


def build_consts(params, adj):
    """Host-side parameter folding. Returns (tensor dict, ed pair lists)."""
    sc = [{k: _np(v) for k, v in d.items()} for d in params['sc']]
    fc = {k: _np(v) for k, v in params['fc'].items()}
    out = {}
    ed_pairs = []

    swl = _np(params['scale_w'])
    sw = np.exp(swl - swl.max())
    sw = sw / sw.sum()
    geo = _sigmoid(_scalar(params['geo_w']))
    ratio = _sigmoid(_scalar(params['hw_ratio']))
    dec_rate = float(np.exp(_scalar(params['log_decay'])))
    decay_vec = np.exp(-dec_rate * np.arange(1, HORIZON + 1, dtype=np.float32))

    # ---- encoder stage 1: fc depthwise (cin=1)+bn1+relu: y1 = relu(W1@xT + b1)
    a1 = float(fc['bn1_g'][0])
    b1 = float(fc['bn1_g'][0] * fc['dw_b'][0] + fc['bn1_b'][0])
    W1 = np.zeros((T, T), np.float32)
    for t in range(T):
        for k in range(KW):
            tp = t + k - 1
            if 0 <= tp < T:
                W1[t, tp] += a1 * fc['dw_w'][0, 0, k]
    out['e1_lhsT'] = W1.T.copy()                        # [64,64]
    out['e1_b'] = np.full((T, 1), b1, np.float32)
    out['e1_nb'] = -out['e1_b']

    # stage 2: fc pointwise 1->16 + bn2 + relu, t-major output chunks
    A2 = fc['bn2_g'] * fc['pw_w'][:, 0, 0]
    B2 = fc['bn2_g'] * fc['pw_b'] + fc['bn2_b']
    e2 = np.zeros((T, NF), np.float32)
    zb = np.zeros(NF, np.float32)
    for f in range(NF):
        t, c = f // FC, f % FC
        e2[t, f] = A2[c]
        zb[f] = B2[c]
    out['e2_lhsT'] = e2.reshape(T, NCH, 128).copy()     # [64, 8, 128]
    out['ez_b'] = zb.reshape(NCH, 128).T.copy()         # [128, 8]
    out['ez_nb'] = -out['ez_b']

    # scale branches
    for i in range(3):
        d = 2 ** i
        g1b, b1b = sc[i]['bn1_g'], sc[i]['bn1_b']
        dw, dwb = sc[i]['dw_w'], sc[i]['dw_b']
        pairs = {}
        ub = np.zeros(NF, np.float32)
        for t in range(T):
            for c in range(FC):
                f = _fidx(t, c)
                ub[f] = g1b[c] * dwb[c] + b1b[c]
                jo, po = f // 128, f % 128
                for k in range(KW):
                    tp = t + d * (k - 1)
                    if 0 <= tp < T:
                        fp = _fidx(tp, c)
                        ji, pi = fp // 128, fp % 128
                        key = (jo, ji)
                        if key not in pairs:
                            pairs[key] = np.zeros((128, 128), np.float32)
                        pairs[key][pi, po] += g1b[c] * dw[c, 0, k]   # lhsT[k_in, m_out]
        keys = sorted(pairs.keys())
        out[f'ed{i}_lhsT'] = np.stack([pairs[k] for k in keys], 1)   # [128, np, 128]
        ed_pairs.append(keys)
        out[f'ed{i}_b'] = ub.reshape(NCH, 128).T.copy()
        out[f'ed{i}_nb'] = -out[f'ed{i}_b']

        g2b, b2b = sc[i]['bn2_g'], sc[i]['bn2_b']
        pw, pwb = sc[i]['pw_w'], sc[i]['pw_b']
        vb = np.zeros(NF, np.float32)
        ep = np.zeros((128, NCH, 128), np.float32)
        for t in range(T):
            for co in range(FC):
                fo = _fidx(t, co)
                vb[fo] = sw[i] * (g2b[co] * pwb[co] + b2b[co])
                jo, po = fo // 128, fo % 128
                for ci in range(FC):
                    fi = _fidx(t, ci)
                    ji, pi = fi // 128, fi % 128
                    assert ji == jo
                    ep[pi, jo, po] = sw[i] * g2b[co] * pw[co, ci, 0]
        out[f'ep{i}_lhsT'] = ep
        out[f'ep{i}_b'] = vb.reshape(NCH, 128).T.copy()
        out[f'ep{i}_nb'] = -out[f'ep{i}_b']

    # ---- proj (columns permuted from c-major reference order to t-major)
    pw_ = _np(params['proj_w'])
    projP = np.zeros((HIDDEN, NF), np.float32)
    for f in range(NF):
        t, c = f // FC, f % FC
        projP[:, f] = pw_[:, c * T + t]
    out['proj_lhsT'] = projP.T.reshape(NCH, 128, HIDDEN).transpose(1, 0, 2).copy()
    out['proj_b'] = _np(params['proj_b']).reshape(2, 128).T.copy()
    out['ln1_g'] = _np(params['ln_g']).reshape(2, 128).T.copy()
    out['ln1_b'] = _np(params['ln_b']).reshape(2, 128).T.copy()
    out['onesH'] = np.full((128, 1), 1.0 / HIDDEN, np.float32)

    # ---- attention
    qkv_w, qkv_b = _np(params['qkv_w']), _np(params['qkv_b'])
    sq = (1.0 - geo) / np.sqrt(HD)
    Wq, bq = qkv_w[:HIDDEN] * sq, qkv_b[:HIDDEN] * sq
    Wk, bk = qkv_w[HIDDEN:2 * HIDDEN], qkv_b[HIDDEN:2 * HIDDEN]
    Wv, bv = qkv_w[2 * HIDDEN:], qkv_b[2 * HIDDEN:]
    Wqk = np.concatenate([Wq, Wk], 0)
    out['qk_lhsT'] = Wqk.T.reshape(2, 128, 2 * HIDDEN).transpose(1, 0, 2).copy()
    out['qk_b'] = np.concatenate([bq, bk]).reshape(4, 128).T.copy()
    out['wv_rhs'] = Wv.T.reshape(2, 128, HIDDEN).transpose(1, 0, 2).copy()
    adjT = _np(adj).T.copy()
    out['adjs'] = (5.0 * geo * adjT).reshape(4, 128, N).transpose(1, 0, 2).copy()
    out['ident'] = np.eye(128, dtype=np.float32)
    sumsel = np.zeros((128, HEADS, HEADS), np.float32)
    for h in range(HEADS):
        sumsel[:, h, h] = 1.0
    out['sumsel'] = sumsel
    ao_w, ao_b = _np(params['ao_w']), _np(params['ao_b'])
    out['ao_lhsT'] = ao_w.T.reshape(2, 128, HIDDEN).transpose(1, 0, 2).copy()
    out['ao_b'] = (ao_b + ao_w @ bv).reshape(2, 128).T.copy()
    out['ln2_g'] = _np(params['an_g']).reshape(2, 128).T.copy()
    out['ln2_b'] = _np(params['an_b']).reshape(2, 128).T.copy()

    # ---- gru
    out['hp_lhsT'] = _np(params['hp_w']).T.reshape(2, 128, H2).transpose(1, 0, 2).copy()
    out['hp_b'] = _np(params['hp_b']).reshape(128, 1).copy()
    out['whh_lhsT'] = _np(params['gru_whh']).T.copy()            # [128, 384]
    wih = _np(params['gru_wih'])[:, 0]
    bih, bhh = _np(params['gru_bih']), _np(params['gru_bhh'])
    op1_w, op1_b = _np(params['op1_w']), _np(params['op1_b'])
    op2_w, op2_b = _np(params['op2_w']), _scalar(params['op2_b'])
    out['gi0_lhsT'] = wih.reshape(1, 3, 128).copy()
    out['gio_lhsT'] = (op2_w[0][:, None, None] * wih.reshape(1, 3, 128)).copy()
    grz0 = (bih + bhh)[:2 * H2].reshape(2, 128).T
    out['grz_b0'] = grz0.copy()
    out['grz_b1'] = (grz0 + (wih[:2 * H2] * op2_b).reshape(2, 128).T).copy()
    out['gn_bhh'] = bhh[2 * H2:].reshape(128, 1).copy()
    out['gn_b0'] = bih[2 * H2:].reshape(128, 1).copy()
    out['gn_b1'] = (bih[2 * H2:] + wih[2 * H2:] * op2_b).reshape(128, 1).copy()
    out['op1_lhsT'] = op1_w.T.copy()                             # [128, 64]
    out['op1_b'] = op1_b.reshape(H4, 1).copy()
    out['op1_nb'] = -out['op1_b']
    oh = np.zeros((H4, HORIZON, HORIZON), np.float32)
    for s_ in range(HORIZON):
        oh[:, s_, s_] = op2_w[0]
    out['op2oh_lhsT'] = oh

    # ---- gate / decay / highway
    out['g1_lhsT'] = _np(params['g1_w']).T.reshape(2, 128, H4).transpose(1, 0, 2).copy()
    out['g1_b'] = _np(params['g1_b']).reshape(H4, 1).copy()
    out['g1_nb'] = -out['g1_b']
    out['g2_lhsT'] = _np(params['g2_w']).T.copy()                # [64, 24]
    out['g2_b'] = _np(params['g2_b']).reshape(HORIZON, 1).copy()
    out['ratio_vec'] = np.full((HORIZON, 1), ratio, np.float32)
    out['dec0_lhsT'] = decay_vec.reshape(1, HORIZON).copy()
    out['negop2b_lhsT'] = np.full((1, HORIZON), -op2_b, np.float32)
    out['rdec_lhsT'] = (ratio * decay_vec).reshape(1, HORIZON).copy()
    hw_w, hw_b = _np(params['hw_w']), _np(params['hw_b'])
    out['hwb_lhsT'] = ((1.0 - ratio) * hw_b).reshape(1, HORIZON).copy()
    out['hw_lhsT'] = ((1.0 - ratio) * hw_w).T.copy()             # [4, 24]
    out['ones_row'] = np.ones((1, R), np.float32)
    return out, ed_pairs


BF16_NAMES = {
    'e1_lhsT', 'e2_lhsT', 'ed0_lhsT', 'ed1_lhsT', 'ed2_lhsT',
    'ep0_lhsT', 'ep1_lhsT', 'ep2_lhsT', 'proj_lhsT', 'sumsel',
}
F32R_NAMES = {
    'onesH', 'qk_lhsT', 'wv_rhs', 'adjs', 'ident', 'ao_lhsT',
    'hp_lhsT', 'whh_lhsT', 'gi0_lhsT', 'gio_lhsT', 'op1_lhsT',
    'op2oh_lhsT', 'g1_lhsT', 'g2_lhsT', 'dec0_lhsT', 'negop2b_lhsT',
    'rdec_lhsT', 'hwb_lhsT', 'hw_lhsT', 'ones_row',
}


def _cdt(name):
    if name in BF16_NAMES:
        return BF16
    if name in F32R_NAMES:
        return F32R
    return F32


def build_nc(const_shapes, ed_pairs):
    nc = bacc.Bacc(target_bir_lowering=False)
    x_c = nc.dram_tensor("x_c", [S, T, N], F32, kind="ExternalInput")
    cd = {name: nc.dram_tensor(name, list(shp), _cdt(name), kind="ExternalInput")
          for name, shp in const_shapes.items()}
    out_d = nc.dram_tensor("out", [S, HORIZON, N], F32, kind="ExternalOutput")

    with tile.TileContext(nc) as tc, contextlib.ExitStack() as ctx:
        cpool = ctx.enter_context(tc.tile_pool(name="consts", bufs=1))
        main = ctx.enter_context(tc.tile_pool(name="main", bufs=1))

        ct = {}
        for name, dram in cd.items():
            tl = cpool.tile(list(const_shapes[name]), _cdt(name), tag=f"c_{name}")
            nc.sync.dma_start(out=tl[:], in_=dram[:])
            ct[name] = tl

        # ---------------- encoder (bf16) ----------------
        x_lastr = main.tile([1, R], F32R, tag="xlast")
        xhw = main.tile([HWIN, R], F32R, tag="xhw")
        v_acc = main.tile([128, NCH, R], BF16, tag="vacc")
        with tc.tile_pool(name="enc_ps", bufs=3, space="PSUM") as eps, \
             tc.tile_pool(name="enc_sb", bufs=1) as esb:
            xT = esb.tile([T, R], F32, tag="xT")
            for b_ in range(S):
                nc.sync.dma_start(out=xT[:, b_ * N:(b_ + 1) * N], in_=x_c[b_])
            xTr = esb.tile([T, R], F32R, tag="xTr")
            nc.vector.tensor_copy(out=xTr[:], in_=xT[:])
            xTb = esb.tile([T, R], BF16, tag="xTb")
            nc.gpsimd.tensor_copy(out=xTb[:], in_=xT[:])
            nc.sync.dma_start(out=x_lastr[:], in_=xTr[T - 1:T, :])
            nc.sync.dma_start(out=xhw[:], in_=xTr[T - HWIN:T, :])
            p_y1 = eps.tile([T, R], F32, tag="ep")
            for f in range(2):
                nc.tensor.matmul(p_y1[:, f * FH:(f + 1) * FH], ct['e1_lhsT'][:],
                                 xTb[:, f * FH:(f + 1) * FH], start=True, stop=True)
            y1 = esb.tile([T, R], BF16, tag="y1")
            nc.vector.tensor_scalar(out=y1[:], in0=p_y1[:], scalar1=ct['e1_nb'][:],
                                    scalar2=ct['e1_b'][:], op0=ALU.max, op1=ALU.add)

            z_sb = esb.tile([128, NCH, R], BF16, tag="z")
            for jo in range(NCH):
                p_z = eps.tile([128, R], F32, tag="ep")
                for f in range(2):
                    nc.tensor.matmul(p_z[:, f * FH:(f + 1) * FH], ct['e2_lhsT'][:, jo, :],
                                     y1[:, f * FH:(f + 1) * FH], start=True, stop=True)
                if jo % 2 == 0:
                    nc.vector.tensor_scalar(out=z_sb[:, jo, :], in0=p_z[:],
                                            scalar1=ct['ez_nb'][:, jo:jo + 1],
                                            scalar2=ct['ez_b'][:, jo:jo + 1],
                                            op0=ALU.max, op1=ALU.add)
                else:
                    nc.scalar.activation(out=z_sb[:, jo, :], in_=p_z[:], func=AF.Relu,
                                         bias=ct['ez_b'][:, jo:jo + 1], scale=1.0)

            for i in range(3):
                pairs = ed_pairs[i]
                u_sb = esb.tile([128, NCH, R], BF16, tag="u")
                for jo in range(NCH):
                    p_u = eps.tile([128, R], F32, tag="ep")
                    ks = [idx for idx, (a_, b_) in enumerate(pairs) if a_ == jo]
                    for f in range(2):
                        for n_, idx in enumerate(ks):
                            ji = pairs[idx][1]
                            nc.tensor.matmul(p_u[:, f * FH:(f + 1) * FH],
                                             ct[f'ed{i}_lhsT'][:, idx, :],
                                             z_sb[:, ji, f * FH:(f + 1) * FH],
                                             start=(n_ == 0), stop=(n_ == len(ks) - 1))
                    if jo % 2 == 0:
                        nc.scalar.activation(out=u_sb[:, jo, :], in_=p_u[:], func=AF.Relu,
                                             bias=ct[f'ed{i}_b'][:, jo:jo + 1], scale=1.0)
                    else:
                        nc.vector.tensor_scalar(out=u_sb[:, jo, :], in0=p_u[:],
                                                scalar1=ct[f'ed{i}_nb'][:, jo:jo + 1],
                                                scalar2=ct[f'ed{i}_b'][:, jo:jo + 1],
                                                op0=ALU.max, op1=ALU.add)
                for jo in range(NCH):
                    p_v = eps.tile([128, R], F32, tag="ep")
                    for f in range(2):
                        nc.tensor.matmul(p_v[:, f * FH:(f + 1) * FH],
                                         ct[f'ep{i}_lhsT'][:, jo, :],
                                         u_sb[:, jo, f * FH:(f + 1) * FH],
                                         start=True, stop=True)
                    if i == 0:
                        if jo % 2 == 0:
                            nc.vector.tensor_scalar(out=v_acc[:, jo, :], in0=p_v[:],
                                                    scalar1=ct['ep0_nb'][:, jo:jo + 1],
                                                    scalar2=ct['ep0_b'][:, jo:jo + 1],
                                                    op0=ALU.max, op1=ALU.add)
                        else:
                            nc.scalar.activation(out=v_acc[:, jo, :], in_=p_v[:],
                                                 func=AF.Relu,
                                                 bias=ct['ep0_b'][:, jo:jo + 1], scale=1.0)
                    else:
                        tmp = esb.tile([128, R], BF16, tag="vtmp")
                        if jo % 2 == 0:
                            nc.scalar.activation(out=tmp[:], in_=p_v[:], func=AF.Relu,
                                                 bias=ct[f'ep{i}_b'][:, jo:jo + 1], scale=1.0)
                        else:
                            nc.vector.tensor_scalar(out=tmp[:], in0=p_v[:],
                                                    scalar1=ct[f'ep{i}_nb'][:, jo:jo + 1],
                                                    scalar2=ct[f'ep{i}_b'][:, jo:jo + 1],
                                                    op0=ALU.max, op1=ALU.add)
                        nc.gpsimd.tensor_tensor(out=v_acc[:, jo, :], in0=v_acc[:, jo, :],
                                                in1=tmp[:], op=ALU.add)

        # ---------------- proj + LN1 + relu ----------------
        featsT = main.tile([128, 2, R], F32R, tag="featsT")
        with tc.tile_pool(name="p1_ps", bufs=1, space="PSUM") as pps, \
             tc.tile_pool(name="p1_sb", bufs=1) as psb:
            p_fp = pps.tile([128, 2, R], F32, tag="pfp")
            for m in range(2):
                for f in range(2):
                    for kc in range(NCH):
                        nc.tensor.matmul(p_fp[:, m, f * FH:(f + 1) * FH],
                                         ct['proj_lhsT'][:, kc, m * 128:(m + 1) * 128],
                                         v_acc[:, kc, f * FH:(f + 1) * FH],
                                         start=(kc == 0), stop=(kc == NCH - 1))
            fp_sb = psb.tile([128, 2, R], F32R, tag="fp")
            x2_sb = psb.tile([128, 2, R], F32R, tag="x2")
            for m in range(2):
                nc.scalar.activation(out=fp_sb[:, m, :], in_=p_fp[:, m, :],
                                     func=AF.Identity, bias=ct['proj_b'][:, m:m + 1],
                                     scale=1.0)
                nc.vector.tensor_tensor(out=x2_sb[:, m, :], in0=fp_sb[:, m, :],
                                        in1=fp_sb[:, m, :], op=ALU.mult)
            p_mean = pps.tile([1, R], F32, tag="pmean")
            p_msq = pps.tile([1, R], F32, tag="pmsq")
            for f in range(2):
                for m in range(2):
                    nc.tensor.matmul(p_mean[:, f * FH:(f + 1) * FH], ct['onesH'][:],
                                     fp_sb[:, m, f * FH:(f + 1) * FH],
                                     start=(m == 0), stop=(m == 1))
                    nc.tensor.matmul(p_msq[:, f * FH:(f + 1) * FH], ct['onesH'][:],
                                     x2_sb[:, m, f * FH:(f + 1) * FH],
                                     start=(m == 0), stop=(m == 1))
            with tc.tile_pool(name="ln1_t", bufs=1) as lnp:
                apply_ln(nc, lnp, p_mean, p_msq, fp_sb, featsT,
                         ct['ln1_g'], ct['ln1_b'], AF.Relu)

        # ---------------- attention ----------------
        feats2T = main.tile([128, 2, R], F32R, tag="feats2T")
        gateT = main.tile([HORIZON, R], F32, tag="gateT")
        with tc.tile_pool(name="at_ps", bufs=1, space="PSUM") as aps, \
             tc.tile_pool(name="at_sb", bufs=1) as asb, \
             tc.tile_pool(name="at_dram", bufs=1, space="DRAM") as adram:
          aoin = asb.tile([128, 2, R], F32R, tag="aoin")
          with tc.tile_pool(name="at_ps1", bufs=2, space="PSUM") as aps1, \
               tc.tile_pool(name="at_sum", bufs=1, space="PSUM") as apsum:
            for b_ in range(S):
                rsl = slice(b_ * N, (b_ + 1) * N)
                p_qk = aps.tile([128, 4, N], F32, tag="a4")
                for o in range(4):
                    for kc in range(2):
                        nc.tensor.matmul(p_qk[:, o, :],
                                         ct['qk_lhsT'][:, kc, o * 128:(o + 1) * 128],
                                         featsT[:, kc, rsl], start=(kc == 0), stop=(kc == 1))
                qk_sb = asb.tile([128, 4, N], F32R, tag="qk")
                for o in range(4):
                    nc.scalar.activation(out=qk_sb[:, o, :], in_=p_qk[:, o, :],
                                         func=AF.Identity, bias=ct['qk_b'][:, o:o + 1],
                                         scale=1.0)
                vT_sb = asb.tile([128, 4, HIDDEN], BF16, tag="vT")
                for mc in range(4):
                    p_vt = aps1.tile([128, HIDDEN], F32, tag="a1")
                    for kc in range(2):
                        nc.tensor.matmul(p_vt[:],
                                         featsT[:, kc, b_ * N + mc * 128:b_ * N + (mc + 1) * 128],
                                         ct['wv_rhs'][:, kc, :], start=(kc == 0), stop=(kc == 1))
                    nc.vector.tensor_copy(out=vT_sb[:, mc, :], in_=p_vt[:])
                E_sb = asb.tile([128, 4, HEADS, N], BF16, tag="E")
                p_sums = apsum.tile([HEADS, N], F32, tag="psums")
                for mc in range(4):
                    p_sc = aps.tile([128, HEADS, N], F32, tag="a4")
                    for h in range(HEADS):
                        po = (h % 2) * 64
                        lhs_k = qk_sb[po:po + 64, 2 + h // 2, mc * 128:(mc + 1) * 128]
                        rhs_q = qk_sb[po:po + 64, h // 2, :]
                        nc.tensor.matmul(p_sc[:, h, :], lhs_k, rhs_q, start=True, stop=False)
                        nc.tensor.matmul(p_sc[:, h, :], ct['ident'][:], ct['adjs'][:, mc, :],
                                         start=False, stop=True)
                    nc.scalar.activation(out=E_sb[:, mc, :, :].rearrange("p h n -> p (h n)"),
                                         in_=p_sc[:].rearrange("p h n -> p (h n)"),
                                         func=AF.Exp, bias=0.0, scale=1.0)
                    for h in range(HEADS):
                        nc.tensor.matmul(p_sums[:], ct['sumsel'][:, h, :], E_sb[:, mc, h, :],
                                         start=(mc == 0 and h == 0), stop=(mc == 3 and h == 3))
                rec = asb.tile([HEADS, N], F32, tag="rec")
                nc.vector.reciprocal_approx_fast(rec[:], p_sums[:])
                rec_d = adram.tile([HEADS, N], F32, tag="recd")
                nc.sync.dma_start(out=rec_d[:], in_=rec[:])
                for h in range(HEADS):
                    rb = asb.tile([64, N], F32, tag="rb")
                    nc.sync.dma_start(out=rb[:],
                                      in_=rec_d[h:h + 1, :].partition_broadcast(64).squeeze(1))
                    p_av = aps1.tile([64, N], F32, tag="a1")
                    for mc in range(4):
                        nc.tensor.matmul(p_av[:], vT_sb[:, mc, h * 64:(h + 1) * 64],
                                         E_sb[:, mc, h, :], start=(mc == 0), stop=(mc == 3))
                    po = (h % 2) * 64
                    nc.vector.tensor_tensor(out=aoin[po:po + 64, h // 2, rsl],
                                            in0=p_av[:], in1=rb[:], op=ALU.mult)
          p_ao = aps.tile([128, 2, R], F32, tag="a4")
          for m in range(2):
              for f in range(2):
                  for kc in range(2):
                      nc.tensor.matmul(p_ao[:, m, f * FH:(f + 1) * FH],
                                       ct['ao_lhsT'][:, kc, m * 128:(m + 1) * 128],
                                       aoin[:, kc, f * FH:(f + 1) * FH],
                                       start=(kc == 0), stop=(kc == 1))
          s2_sb = asb.tile([128, 2, R], F32R, tag="s2")
          x2b_sb = asb.tile([128, 2, R], F32R, tag="x2b")
          for m in range(2):
              nc.vector.scalar_tensor_tensor(out=s2_sb[:, m, :], in0=p_ao[:, m, :],
                                             scalar=ct['ao_b'][:, m:m + 1],
                                             in1=featsT.bitcast(F32)[:, m, :],
                                             op0=ALU.add, op1=ALU.add)
              nc.vector.tensor_tensor(out=x2b_sb[:, m, :], in0=s2_sb[:, m, :],
                                      in1=s2_sb[:, m, :], op=ALU.mult)
          with tc.tile_pool(name="at_st", bufs=1, space="PSUM") as apst:
            p_mean2 = apst.tile([1, R], F32, tag="a1")
            p_msq2 = apst.tile([1, R], F32, tag="a1b")
            for f in range(2):
                for m in range(2):
                    nc.tensor.matmul(p_mean2[:, f * FH:(f + 1) * FH], ct['onesH'][:],
                                     s2_sb[:, m, f * FH:(f + 1) * FH],
                                     start=(m == 0), stop=(m == 1))
                    nc.tensor.matmul(p_msq2[:, f * FH:(f + 1) * FH], ct['onesH'][:],
                                     x2b_sb[:, m, f * FH:(f + 1) * FH],
                                     start=(m == 0), stop=(m == 1))
            with tc.tile_pool(name="ln2_t", bufs=1) as lnp:
                apply_ln(nc, lnp, p_mean2, p_msq2, s2_sb, feats2T,
                         ct['ln2_g'], ct['ln2_b'], AF.Identity)

        # ---------------- gate + h0 ----------------
        hA = main.tile([H2, R], F32R, tag="hA")
        hB = main.tile([H2, R], F32R, tag="hB")
        o1_sb = main.tile([H4, R], F32R, tag="o1")
        with tc.tile_pool(name="g_ps", bufs=1, space="PSUM") as gps, \
             tc.tile_pool(name="g_sb", bufs=1) as gsb:
            p_g1 = gps.tile([H4, R], F32, tag="pg1")
            for f in range(2):
                for kc in range(2):
                    nc.tensor.matmul(p_g1[:, f * FH:(f + 1) * FH], ct['g1_lhsT'][:, kc, :],
                                     feats2T[:, kc, f * FH:(f + 1) * FH],
                                     start=(kc == 0), stop=(kc == 1))
            g1o = gsb.tile([H4, R], F32R, tag="g1o")
            nc.vector.tensor_scalar(out=g1o[:], in0=p_g1[:], scalar1=ct['g1_nb'][:],
                                    scalar2=ct['g1_b'][:], op0=ALU.max, op1=ALU.add)
            p_g2 = gps.tile([HORIZON, R], F32, tag="pg2")
            for f in range(2):
                nc.tensor.matmul(p_g2[:, f * FH:(f + 1) * FH], ct['g2_lhsT'][:],
                                 g1o[:, f * FH:(f + 1) * FH], start=True, stop=True)
            nc.scalar.activation(out=gateT[:], in_=p_g2[:], func=AF.Sigmoid,
                                 bias=ct['g2_b'][:], scale=1.0)
            p_h0 = gps.tile([H2, R], F32, tag="ph0")
            for f in range(2):
                for kc in range(2):
                    nc.tensor.matmul(p_h0[:, f * FH:(f + 1) * FH], ct['hp_lhsT'][:, kc, :],
                                     feats2T[:, kc, f * FH:(f + 1) * FH],
                                     start=(kc == 0), stop=(kc == 1))
            nc.scalar.activation(out=hA[:], in_=p_h0[:], func=AF.Identity,
                                 bias=ct['hp_b'][:], scale=1.0)

        # ---------------- GRU over horizon ----------------
        with tc.tile_pool(name="r_pp", bufs=1, space="PSUM") as rpp:
            p_preds = rpp.tile([HORIZON, R], F32, tag="ppreds")
            with tc.tile_pool(name="r_ps", bufs=1, space="PSUM") as rps, \
                 tc.tile_pool(name="r_pa", bufs=2, space="PSUM") as rpa, \
                 tc.tile_pool(name="r_sb", bufs=2) as rsb:
                for s_ in range(HORIZON):
                    h_in = hA if s_ % 2 == 0 else hB
                    h_out = hB if s_ % 2 == 0 else hA
                    bsel = 'grz_b0' if s_ == 0 else 'grz_b1'
                    nbse = 'gn_b0' if s_ == 0 else 'gn_b1'
                    for f in range(2):
                        fs = slice(f * FH, (f + 1) * FH)
                        p_g = rps.tile([128, 4, FH], F32, tag="pg")
                        for g in range(3):
                            # slice g==2 (ghn) is whh-only -> complete group
                            nc.tensor.matmul(p_g[:, g, :],
                                             ct['whh_lhsT'][:, g * 128:(g + 1) * 128],
                                             h_in[:, fs], start=True, stop=(g == 2))
                        for g in range(3):
                            tgt = p_g[:, g, :] if g < 2 else p_g[:, 3, :]
                            st = (g == 2)       # gin slice starts its own group
                            if s_ == 0:
                                nc.tensor.matmul(tgt, ct['gi0_lhsT'][:, g, :],
                                                 x_lastr[:, fs], start=st, stop=True)
                            else:
                                nc.tensor.matmul(tgt, ct['gio_lhsT'][:, g, :],
                                                 o1_sb[:, fs], start=st, stop=True)
                        r_t = rsb.tile([H2, FH], F32R, tag="r")
                        z_t = rsb.tile([H2, FH], F32R, tag="z")
                        n_t = rsb.tile([H2, FH], F32R, tag="n")
                        ta_t = rsb.tile([H2, FH], F32R, tag="ta")
                        tb_t = rsb.tile([H2, FH], F32R, tag="tb")
                        q_t = rsb.tile([H2, FH], F32R, tag="q")
                        u_t = rsb.tile([H2, FH], F32R, tag="u")
                        w_t = rsb.tile([H2, FH], F32R, tag="w")
                        nc.scalar.activation(out=r_t[:], in_=p_g[:, 0, :], func=AF.Sigmoid,
                                             bias=ct[bsel][:, 0:1], scale=1.0)
                        nc.scalar.activation(out=z_t[:], in_=p_g[:, 1, :], func=AF.Sigmoid,
                                             bias=ct[bsel][:, 1:2], scale=1.0)
                        nc.vector.scalar_tensor_tensor(out=ta_t[:], in0=p_g[:, 2, :],
                                                       scalar=ct['gn_bhh'][:], in1=r_t[:],
                                                       op0=ALU.add, op1=ALU.mult)
                        nc.vector.scalar_tensor_tensor(out=tb_t[:], in0=p_g[:, 3, :],
                                                       scalar=ct[nbse][:], in1=ta_t[:],
                                                       op0=ALU.add, op1=ALU.add)
                        nc.scalar.activation(out=n_t[:], in_=tb_t[:], func=AF.Tanh,
                                             bias=0.0, scale=1.0)
                        nc.gpsimd.tensor_tensor(out=q_t[:], in0=z_t[:], in1=h_in[:, fs],
                                                op=ALU.mult)
                        nc.vector.tensor_tensor(out=u_t[:], in0=z_t[:], in1=n_t[:],
                                                op=ALU.mult)
                        nc.vector.tensor_tensor(out=w_t[:], in0=n_t[:], in1=u_t[:],
                                                op=ALU.subtract)
                        nc.gpsimd.tensor_tensor(out=h_out[:, fs], in0=w_t[:], in1=q_t[:],
                                                op=ALU.add)
                        p_o1 = rpa.tile([H4, FH], F32, tag="paux")
                        nc.tensor.matmul(p_o1[:], ct['op1_lhsT'][:], h_out[:, fs],
                                         start=True, stop=True)
                        nc.vector.tensor_scalar(out=o1_sb[:, fs], in0=p_o1[:],
                                                scalar1=ct['op1_nb'][:],
                                                scalar2=ct['op1_b'][:],
                                                op0=ALU.max, op1=ALU.add)
                        nc.tensor.matmul(p_preds[:, fs], ct['op2oh_lhsT'][:, s_, :],
                                         o1_sb[:, fs],
                                         start=(s_ == 0), stop=(s_ == HORIZON - 1))

            # ---------------- decay / highway / final ----------------
            with tc.tile_pool(name="f_ps", bufs=1, space="PSUM") as fps, \
                 tc.tile_pool(name="f_sb", bufs=1) as fsb:
                p_dec = fps.tile([HORIZON, R], F32, tag="pdec")
                p_base = fps.tile([HORIZON, R], F32, tag="pbase")
                onr = ct['ones_row']
                for f in range(2):
                    fs = slice(f * FH, (f + 1) * FH)
                    nc.tensor.matmul(p_dec[:, fs], ct['dec0_lhsT'][:], x_lastr[:, fs],
                                     start=True, stop=False)
                    nc.tensor.matmul(p_dec[:, fs], ct['negop2b_lhsT'][:], onr[:, fs],
                                     start=False, stop=True)
                    nc.tensor.matmul(p_base[:, fs], ct['rdec_lhsT'][:], x_lastr[:, fs],
                                     start=True, stop=False)
                    nc.tensor.matmul(p_base[:, fs], ct['hwb_lhsT'][:], onr[:, fs],
                                     start=False, stop=False)
                    nc.tensor.matmul(p_base[:, fs], ct['hw_lhsT'][:],
                                     xhw[:, fs], start=False, stop=True)
                dec_sb = fsb.tile([HORIZON, R], F32, tag="dec")
                nc.vector.tensor_copy(out=dec_sb[:], in_=p_dec[:])
                u1 = fsb.tile([HORIZON, R], F32, tag="u1")
                nc.vector.tensor_tensor(out=u1[:], in0=p_preds[:], in1=dec_sb[:],
                                        op=ALU.subtract)
                u2 = fsb.tile([HORIZON, R], F32, tag="u2")
                nc.vector.tensor_tensor(out=u2[:], in0=gateT[:], in1=u1[:], op=ALU.mult)
                finalT = fsb.tile([HORIZON, R], F32, tag="finalT")
                nc.vector.scalar_tensor_tensor(out=finalT[:], in0=u2[:],
                                               scalar=ct['ratio_vec'][:], in1=p_base[:],
                                               op0=ALU.mult, op1=ALU.add)
                for b_ in range(S):
                    nc.sync.dma_start(out=out_d[b_], in_=finalT[:, b_ * N:(b_ + 1) * N])

    nc.finalize()
    return nc


def apply_ln(nc, pool, p_mean, p_msq, x_sb, out_t, g_ap, b_ap, func):
    """LayerNorm over the partition (feature) axis.

    p_mean/p_msq = mean and mean(x^2) (already 1/H-scaled). Writes out_t f32r.
    """
    stat_sb = pool.tile([1, R], F32, tag="lnS")
    nc.vector.tensor_copy(out=stat_sb[:], in_=p_mean[:])
    msq = pool.tile([1, R], F32, tag="lnB")
    nc.vector.tensor_tensor(out=msq[:], in0=stat_sb[:], in1=stat_sb[:],
                            op=ALU.mult)
    ve = pool.tile([1, R], F32, tag="lnC")
    nc.vector.scalar_tensor_tensor(out=ve[:], in0=p_msq[:], scalar=EPS, in1=msq[:],
                                   op0=ALU.add, op1=ALU.subtract)
    lnv = pool.tile([1, R], F32, tag="lnB")
    nc.scalar.activation(out=lnv[:], in_=ve[:], func=AF.Ln, bias=0.0, scale=1.0)
    istd = pool.tile([1, R], F32, tag="lnC")
    nc.scalar.activation(out=istd[:], in_=lnv[:], func=AF.Exp, bias=0.0, scale=-0.5)
    mi = pool.tile([1, R], F32, tag="lnB")
    nc.vector.scalar_tensor_tensor(out=mi[:], in0=stat_sb[:], scalar=-1.0,
                                   in1=istd[:], op0=ALU.mult, op1=ALU.mult)
    istd_b = pool.tile([128, R], F32, tag="lnistdb")
    nc.gpsimd.partition_broadcast(istd_b[:], istd[:])
    mi_b = pool.tile([128, R], F32, tag="lnmib")
    nc.gpsimd.partition_broadcast(mi_b[:], mi[:])
    for m in range(2):
        t1 = pool.tile([128, R], F32, tag="lnt1")
        nc.vector.scalar_tensor_tensor(out=t1[:], in0=x_sb.bitcast(F32)[:, m, :],
                                       scalar=g_ap[:, m:m + 1], in1=istd_b[:],
                                       op0=ALU.mult, op1=ALU.mult)
        t2 = pool.tile([128, R], F32, tag="lnt2")
        nc.vector.scalar_tensor_tensor(out=t2[:], in0=mi_b[:], scalar=g_ap[:, m:m + 1],
                                       in1=t1[:], op0=ALU.mult, op1=ALU.add)
        nc.scalar.activation(out=out_t[:, m, :], in_=t2[:], func=func,
                             bias=b_ap[:, m:m + 1], scale=1.0)


_CACHE = {}


def _to_dev(name, arr):
    if name in BF16_NAMES:
        import ml_dtypes
        return np.ascontiguousarray(arr.astype(ml_dtypes.bfloat16))
    return np.ascontiguousarray(arr.astype(np.float32))


def kernel(x, adj, params):
    x = _np(x)
    consts, ed_pairs = build_consts(params, adj)
    if 'nc' not in _CACHE:
        shapes = {k: v.shape for k, v in consts.items()}
        _CACHE['nc'] = build_nc(shapes, ed_pairs)
    nc = _CACHE['nc']
    dev_consts = {k: _to_dev(k, v) for k, v in consts.items()}
    in_maps = []
    for c in range(NCORES):
        m = dict(dev_consts)
        m['x_c'] = np.ascontiguousarray(x[c * S:(c + 1) * S])
        in_maps.append(m)
    res = run_bass_kernel_spmd(nc, in_maps, list(range(NCORES)))
    out = np.concatenate([res.results[c]['out'] for c in range(NCORES)], axis=0)
    return (out, np.float32(0.0))
